# revision 43
# baseline (speedup 1.0000x reference)
"""GPT-2 attention (B=2, S=2048, D=1024, H=16) on 8 TRN2 NeuronCores.

Sharding: 2-way data parallel over batch x 4-way tensor parallel over heads.
Core c handles batch b = c // 4 and heads 4g..4g+3 where g = c % 4.

Per-core kernel (all matmul inputs bf16, fp32 PSUM accumulation):
  1. QKV^T projection: Q^T, K^T computed in [head_dim, seq] layout
     (lhsT = W tiles, rhs = x^T tiles); V computed in natural [seq, head_dim]
     layout (lhsT = x^T tiles, rhs = Wv) with a ones column appended.
     Production is interleaved with attention chunk-by-chunk so the QKV
     matmuls overlap the exp (ScalarE) of earlier chunks.
  2. Per (chunk, head) unit: scores^T[sk, sq] = K^T_tile.T @ Q^T (one K=64
     matmul per sk tile, restricted to valid causal columns), causal mask
     applied by accumulating a -1e9 upper-triangle tile into the scores PSUM
     via an identity matmul, then exp on ScalarE (scale = 1/sqrt(64))
     directly PSUM->SBUF bf16.
  3. AV: O_aug^T[65, sq] = V_aug.T @ P^T accumulated over sk tiles; row 64
     (from the ones column of V_aug) is the softmax denominator.
  4. Normalization: denominator row is bounced through DRAM to re-partition
     [1, 512] -> [128, 4] for a full-width reciprocal, broadcast across 64
     partitions with a K=1 matmul, and multiplied into O^T (the division
     commutes out of the output projection's contraction). Emitted a few
     units later so the chain latency hides behind other matmuls.
  5. Output projection: y_partial[sq, 1024] = O_scaled^T.T @ Wproj_shard.
     The first 3/4 (which only need already-normalized chunks) are emitted
     before the final norms to cover their latency.

Host: x[b].T and weight shards pre-cast to bf16; the 4 per-batch partials
are summed on host (row-split matmul unshard) and bproj added.
"""

import numpy as np
import ml_dtypes

import concourse.bass as bass
import concourse.mybir as mybir
import concourse.tile as tile
from concourse import bacc
from concourse.bass_utils import run_bass_kernel_spmd
from concourse.masks import make_identity

BF16 = ml_dtypes.bfloat16

B, S, D, H = 2, 2048, 1024, 16
HD = D // H            # 64
NH = 4                 # heads per core
JQ = NH * HD           # 256 q (or k, or v) columns per core
P = 128
SC = 512               # seq chunk (matmul free dim / PSUM bank)
NSC = S // SC          # 4
NST = S // P           # 16 seq tiles
NDC = D // P           # 8 contraction chunks over model dim
SCALE = 1.0 / np.sqrt(np.float32(HD))  # 0.125
NEG = -1.0e9

_COMPILED = {}


def build(has_qkv_bias: bool):
    f32 = mybir.dt.float32
    bf16 = mybir.dt.bfloat16
    nc = bacc.Bacc()

    xT = nc.declare_dram_parameter("xT", [D, S], bf16, isOutput=False)
    w = nc.declare_dram_parameter("w", [D, 3 * JQ], bf16, isOutput=False)
    wp = nc.declare_dram_parameter("wp", [JQ, D], bf16, isOutput=False)
    if has_qkv_bias:
        bqkv = nc.declare_dram_parameter("bqkv", [1, 3 * JQ], bf16, isOutput=False)
    y = nc.declare_dram_parameter("y", [S, D], bf16, isOutput=True)

    with tile.TileContext(nc) as tc:
        with (
            tc.tile_pool(name="const", bufs=1) as const,
            tc.tile_pool(name="inp", bufs=1) as inp,
            tc.tile_pool(name="qkv", bufs=1) as qkv,
            tc.tile_pool(name="ptp", bufs=6) as ptp,
            tc.tile_pool(name="ps", bufs=4, space="PSUM") as psp,
            tc.tile_pool(name="drp", bufs=1, space="DRAM") as drp,
        ):
            # ---- constants ----
            # -1e9 on the strict upper triangle (j < p), 0 elsewhere: added
            # into diagonal scores blocks via an identity matmul
            trineg = const.tile([P, P], bf16)
            nc.gpsimd.memset(trineg[:], 0.0)
            nc.gpsimd.affine_select(
                out=trineg[:],
                in_=trineg[:],
                compare_op=mybir.AluOpType.is_ge,
                fill=NEG,
                base=0,
                pattern=[[1, P]],
                channel_multiplier=-1,
            )
            ident = const.tile([P, P], bf16)
            make_identity(nc, ident)
            # ones rows: row 0 feeds bias matmuls, row 64 feeds the
            # denominator-broadcast matmul (lhsT/rhs must share base partition)
            ones = const.tile([P, SC], bf16)
            nc.gpsimd.memset(ones[:], 1.0)

            # ---- load inputs, split so early consumers start immediately ----
            # w tiles 0,1: Q columns; 2,3: K columns; V columns in one tile
            wt = [inp.tile([P, NDC, P], bf16, name=f"wt{j}") for j in range(4)]
            wv_sb = inp.tile([P, NDC, JQ], bf16)
            xc = [inp.tile([P, NDC, SC], bf16, name=f"xc{c}") for c in range(NSC)]

            def dma_w(j):
                nc.sync.dma_start(
                    out=wt[j][:],
                    in_=w[:, j * P:(j + 1) * P].rearrange("(a p) j2 -> p a j2", p=P),
                )

            def dma_x(c):
                nc.sync.dma_start(
                    out=xc[c][:],
                    in_=xT[:, c * SC:(c + 1) * SC].rearrange("(a p) s -> p a s", p=P),
                )

            dma_w(2)
            dma_w(0)
            # first chunk split in two so the first QKV matmul starts sooner
            nc.sync.dma_start(
                out=xc[0][:, 0:4, :],
                in_=xT[0:D // 2, 0:SC].rearrange("(a p) s -> p a s", p=P),
            )
            nc.sync.dma_start(
                out=xc[0][:, 4:NDC, :],
                in_=xT[D // 2:D, 0:SC].rearrange("(a p) s -> p a s", p=P),
            )
            nc.sync.dma_start(
                out=wv_sb[:],
                in_=w[:, 2 * JQ:3 * JQ].rearrange("(a p) j2 -> p a j2", p=P),
            )
            dma_w(3)
            dma_w(1)
            for c in range(1, NSC):
                dma_x(c)
            wp_sb = inp.tile([P, JQ // P, D], bf16)
            nc.sync.dma_start(out=wp_sb[:], in_=wp[:].rearrange("(a p) j -> p a j", p=P))
            if has_qkv_bias:
                b_sb = inp.tile([1, 3 * JQ], bf16)
                nc.sync.dma_start(out=b_sb[:], in_=bqkv[:])

            qT = qkv.tile([P, 2, S], bf16)  # partitions: head pair (h%2)*64 + hd
            kT = qkv.tile([P, 2, S], bf16)
            v_sb = qkv.tile([P, NST * NH, HD + 1], bf16)
            nc.vector.memset(v_sb[:, :, HD:HD + 1], 1.0)
            oT = qkv.tile([P, 2, S], bf16)
            dr_s = drp.tile([NH, NSC, 1, SC], f32)
            dr_r = drp.tile([NH, NSC, 1, SC], bf16)

            def emit_qk_chunk(jt, c):
                # one [128, SC] chunk of Q^T (jt 0,1) or K^T (jt 2,3)
                dest, jl = (qT, jt) if jt < 2 else (kT, jt - 2)
                ps_qkv = psp.tile([P, SC], f32, tag="ps", name="ps_qkv")
                for a in range(NDC):
                    nc.tensor.matmul(
                        ps_qkv[:],
                        lhsT=wt[jt][:, a, :],
                        rhs=xc[c][:, a, :],
                        start=(a == 0),
                        stop=(a == NDC - 1) and not has_qkv_bias,
                    )
                if has_qkv_bias:
                    nc.tensor.matmul(
                        ps_qkv[:],
                        lhsT=b_sb[0:1, jt * P:(jt + 1) * P],
                        rhs=ones[0:1, :SC],
                        start=False,
                        stop=True,
                    )
                nc.vector.tensor_copy(dest[:, jl, c * SC:(c + 1) * SC], ps_qkv[:])

            def emit_v_tile(t):
                # V rows for seq tile t, all 4 heads, with the ones column
                ps_v = psp.tile([P, SC], f32, tag="ps", name="ps_v")
                for a in range(NDC):
                    nc.tensor.matmul(
                        ps_v[:, 0:JQ],
                        lhsT=xc[t // 4][:, a, (t % 4) * P:(t % 4 + 1) * P],
                        rhs=wv_sb[:, a, :],
                        start=(a == 0),
                        stop=(a == NDC - 1) and not has_qkv_bias,
                    )
                if has_qkv_bias:
                    nc.tensor.matmul(
                        ps_v[:, 0:JQ],
                        lhsT=ones[0:1, 0:P],
                        rhs=b_sb[0:1, 2 * JQ:3 * JQ],
                        start=False,
                        stop=True,
                    )
                nc.vector.tensor_copy(
                    v_sb[:, t * NH:(t + 1) * NH, 0:HD],
                    ps_v[:, 0:JQ].rearrange("p (h d) -> p h d", d=HD),
                )

            # per-(head, chunk) softmax-denominator reciprocal chains; the
            # normalization (bcast matmul + O^T scale) is emitted a few units
            # later so the chain latency never stalls the PE
            norm_pend = []

            def emit_norm():
                h, c, recipst = norm_pend.pop(0)
                jl, po = h // 2, (h % 2) * HD
                ps_bc = psp.tile([P, SC], f32, tag="ps", name="ps_bc")
                nc.tensor.matmul(
                    ps_bc[0:HD, :],
                    lhsT=ones[64:65, 0:HD],
                    rhs=recipst[64:65, :],
                    start=True,
                    stop=True,
                )
                nc.vector.tensor_mul(
                    oT[po:po + HD, jl, c * SC:(c + 1) * SC],
                    oT[po:po + HD, jl, c * SC:(c + 1) * SC],
                    ps_bc[0:HD, :],
                )

            def emit_scores(c, h):
                jl, po = h // 2, (h % 2) * HD
                nv = min(4 * (c + 1), NST)  # valid sk tiles: t*128 <= c*512+511
                pt = ptp.tile([P, NST, SC], bf16, tag="pt", name="pt")
                for t in range(nv):
                    # first valid column within this sq chunk (causal)
                    coff = max(0, t * P - c * SC)
                    diag = t >= 4 * c
                    ps_sc = psp.tile([P, SC], f32, tag="ps_sc", name="ps_sc", bufs=4)
                    nc.tensor.matmul(
                        ps_sc[:, coff:],
                        lhsT=kT[po:po + HD, jl, t * P:(t + 1) * P],
                        rhs=qT[po:po + HD, jl, c * SC + coff:(c + 1) * SC],
                        start=True,
                        stop=not diag,
                    )
                    if diag:  # add -1e9 above the diagonal: psum += I.T @ trineg
                        nc.tensor.matmul(
                            ps_sc[:, coff:coff + P],
                            lhsT=ident[:],
                            rhs=trineg[:],
                            start=False,
                            stop=True,
                        )
                    nc.scalar.activation(
                        pt[:, t, coff:], ps_sc[:, coff:],
                        mybir.ActivationFunctionType.Exp, scale=float(SCALE),
                    )
                return (c, h, pt)

            def emit_av(state, direct_recip=False):
                c, h, pt = state
                jl, po = h // 2, (h % 2) * HD
                nv = min(4 * (c + 1), NST)
                ps_av = psp.tile([P, SC], f32, tag="ps", name="ps_av")
                for t in range(nv):
                    coff = max(0, t * P - c * SC)
                    nc.tensor.matmul(
                        ps_av[0:HD + 1, coff:],
                        lhsT=v_sb[:, t * NH + h, :],
                        rhs=pt[:, t, coff:],
                        start=(t == 0),
                        stop=(t == nv - 1),
                    )
                nc.vector.tensor_copy(
                    oT[po:po + HD, jl, c * SC:(c + 1) * SC], ps_av[0:HD, :]
                )

                # denominator reciprocal: stage the sums row, bounce through
                # DRAM to re-partition [1, SC] -> [128, SC/128] so the
                # reciprocal runs on all 128 lanes instead of one; the last
                # units take the short single-lane path instead (lower latency,
                # nothing left to hide the DMA chain behind)
                recipst = ptp.tile([P, SC], bf16, tag="recipst",
                                   name="recipst", bufs=8)
                if direct_recip:
                    with nc.allow_low_precision(reason="bf16 softmax denom recip"):
                        nc.vector.reciprocal(recipst[64:65, :], ps_av[HD:HD + 1, :])
                else:
                    sumst = ptp.tile([P, SC], f32, tag="sumst", name="sumst", bufs=3)
                    rsc = ptp.tile([P, SC // P], f32, tag="rsc", name="rsc", bufs=3)
                    rscb = ptp.tile([P, SC // P], bf16, tag="rscb", name="rscb",
                                    bufs=3)
                    nc.vector.tensor_copy(sumst[64:65, :], ps_av[HD:HD + 1, :])
                    nc.sync.dma_start(out=dr_s[h, c], in_=sumst[64:65, :])
                    nc.sync.dma_start(
                        out=rsc[:], in_=dr_s[h, c].rearrange("x (p k) -> (x p) k", p=P)
                    )
                    with nc.allow_low_precision(reason="bf16 softmax denom recip"):
                        nc.vector.reciprocal(rscb[:], rsc[:])
                    nc.sync.dma_start(
                        out=dr_r[h, c].rearrange("x (p k) -> (x p) k", p=P),
                        in_=rscb[:],
                    )
                    nc.sync.dma_start(out=recipst[64:65, :], in_=dr_r[h, c])
                norm_pend.append((h, c, recipst))
                while len(norm_pend) > 6:
                    emit_norm()

            def emit_proj(st, jc):
                ps_y = psp.tile([P, SC], f32, tag="ps", name="ps_y")
                for cc in range(2):
                    nc.tensor.matmul(
                        ps_y[:],
                        lhsT=oT[:, cc, st * P:(st + 1) * P],
                        rhs=wp_sb[:, cc, jc * SC:(jc + 1) * SC],
                        start=(cc == 0),
                        stop=(cc == 1),
                    )
                y_sb = ptp.tile([P, SC], bf16, tag="ysb", name="y_sb", bufs=4)
                nc.vector.tensor_copy(y_sb[:], ps_y[:])
                nc.sync.dma_start(
                    out=y[st * P:(st + 1) * P, jc * SC:(jc + 1) * SC], in_=y_sb[:]
                )

            # ---- main schedule ----
            # Chunk 0: produce only what each unit needs, then start it, so
            # the first exp lands on ScalarE as early as possible.
            def prod_thunks(c):
                if c >= NSC:
                    return []
                th = [lambda: emit_qk_chunk(2, c), lambda: emit_qk_chunk(0, c)]
                th += [lambda t=t: emit_v_tile(t) for t in range(4 * c, 4 * c + 4)]
                th += [lambda: emit_qk_chunk(3, c), lambda: emit_qk_chunk(1, c)]
                return th

            emit_qk_chunk(2, 0)
            emit_qk_chunk(0, 0)
            for t in range(4):
                emit_v_tile(t)
            pq = prod_thunks(1)

            def pop2():
                for _ in range(2):
                    if pq:
                        pq.pop(0)()

            # AV for each unit is emitted one unit late so the exps of a unit
            # get a whole unit of matmul time before AV consumes them
            av_pend = []

            def flush_av(keep):
                while len(av_pend) > keep:
                    st_ = av_pend.pop(0)
                    emit_av(st_, direct_recip=(st_[0] == NSC - 1 and st_[1] >= 2))

            av_pend.append(emit_scores(0, 0))
            emit_qk_chunk(3, 0)
            av_pend.append(emit_scores(0, 1))
            flush_av(1)
            emit_qk_chunk(1, 0)
            pop2()
            av_pend.append(emit_scores(0, 2))
            flush_av(1)
            pop2()
            av_pend.append(emit_scores(0, 3))
            flush_av(1)
            while pq:
                pq.pop(0)()

            # Chunks 1..3: attention units interleaved with chunk c+1's
            # production and (from chunk 2 on) with the projection of chunk
            # c-2, whose normalization completed during chunk c-1.
            for c in range(1, NSC):
                pq = prod_thunks(c + 1)
                for h in range(NH):
                    av_pend.append(emit_scores(c, h))
                    flush_av(1)
                    if c >= 2:
                        while norm_pend and norm_pend[0][1] <= c - 2:
                            emit_norm()
                        st = 4 * (c - 2) + h
                        emit_proj(st, 0)
                        emit_proj(st, 1)
                    if c == NSC - 1 and h >= 2:
                        while norm_pend and norm_pend[0][1] <= c - 1:
                            emit_norm()
                        for st in (8 + 2 * (h - 2), 9 + 2 * (h - 2)):
                            emit_proj(st, 0)
                            emit_proj(st, 1)
                    pop2()
                while pq:
                    pq.pop(0)()
            flush_av(0)

            # tail: chunk-3 projections after its norms
            while norm_pend:
                emit_norm()
            for st in range(12, NST):
                for jc in range(2):
                    emit_proj(st, jc)

    nc.compile()
    return nc


def get_compiled(has_qkv_bias: bool):
    key = bool(has_qkv_bias)
    if key not in _COMPILED:
        _COMPILED[key] = build(key)
    return _COMPILED[key]


def make_in_maps(x, Wqkv, bqkv, Wproj):
    has_bias = bool(np.any(bqkv))
    xTs = [np.ascontiguousarray(x[b].T).astype(BF16) for b in range(B)]
    in_maps = []
    for c in range(8):
        b, g = c // 4, c % 4
        sl = slice(g * JQ, (g + 1) * JQ)
        wshard = np.concatenate(
            [Wqkv[:, sl], Wqkv[:, D + g * JQ:D + (g + 1) * JQ],
             Wqkv[:, 2 * D + g * JQ:2 * D + (g + 1) * JQ]], axis=1
        ).astype(BF16)
        m = {
            "xT": xTs[b],
            "w": np.ascontiguousarray(wshard),
            "wp": np.ascontiguousarray(Wproj[sl]).astype(BF16),
        }
        if has_bias:
            bshard = np.concatenate(
                [bqkv[sl], bqkv[D + g * JQ:D + (g + 1) * JQ],
                 bqkv[2 * D + g * JQ:2 * D + (g + 1) * JQ]]
            ).astype(BF16)
            m["bqkv"] = np.ascontiguousarray(bshard[None, :])
        in_maps.append(m)
    return has_bias, in_maps


def run(x, Wqkv, bqkv, Wproj, bproj, trace=False):
    has_bias, in_maps = make_in_maps(x, Wqkv, bqkv, Wproj)
    nc = get_compiled(has_bias)
    res = run_bass_kernel_spmd(nc, in_maps, core_ids=list(range(8)), trace=trace)
    out = np.zeros((B, S, D), np.float32)
    for c in range(8):
        out[c // 4] += res.results[c]["y"].astype(np.float32)
    out += bproj.astype(np.float32)
    return out, res


def kernel(x, Wqkv, bqkv, Wproj, bproj):
    x = np.asarray(x, np.float32)
    Wqkv = np.asarray(Wqkv, np.float32)
    bqkv = np.asarray(bqkv, np.float32)
    Wproj = np.asarray(Wproj, np.float32)
    bproj = np.asarray(bproj, np.float32)
    out, _ = run(x, Wqkv, bqkv, Wproj, bproj, trace=False)
    return out


# revision 44
# speedup vs baseline: 1.0717x; 1.0717x over previous
"""GPT-2 attention (B=2, S=2048, D=1024, H=16) on 8 TRN2 NeuronCores.

Sharding: 2-way data parallel over batch x 4-way tensor parallel over heads.
Core c handles batch b = c // 4 and heads 4g..4g+3 where g = c % 4.

Per-core kernel (all matmul inputs bf16, fp32 PSUM accumulation):
  1. QKV^T projection: Q^T, K^T computed in [head_dim, seq] layout
     (lhsT = W tiles, rhs = x^T tiles); V computed in natural [seq, head_dim]
     layout (lhsT = x^T tiles, rhs = Wv) with a ones column appended.
     Production is interleaved with attention chunk-by-chunk so the QKV
     matmuls overlap the exp (ScalarE) of earlier chunks.
  2. Per (chunk, head) unit: scores^T[sk, sq] = K^T_tile.T @ Q^T (one K=64
     matmul per sk tile, restricted to valid causal columns), causal mask
     applied by accumulating a -1e9 upper-triangle tile into the scores PSUM
     via an identity matmul, then exp on ScalarE (scale = 1/sqrt(64))
     directly PSUM->SBUF bf16.
  3. AV: O_aug^T[65, sq] = V_aug.T @ P^T accumulated over sk tiles; row 64
     (from the ones column of V_aug) is the softmax denominator.
  4. Normalization: denominator row is bounced through DRAM to re-partition
     [1, 512] -> [128, 4] for a full-width reciprocal, broadcast across 64
     partitions with a K=1 matmul, and multiplied into O^T (the division
     commutes out of the output projection's contraction). Emitted a few
     units later so the chain latency hides behind other matmuls.
  5. Output projection: y_partial[sq, 1024] = O_scaled^T.T @ Wproj_shard.
     The first 3/4 (which only need already-normalized chunks) are emitted
     before the final norms to cover their latency.

Host: x[b].T and weight shards pre-cast to bf16; the 4 per-batch partials
are summed on host (row-split matmul unshard) and bproj added.
"""

import numpy as np
import ml_dtypes

import concourse.bass as bass
import concourse.mybir as mybir
import concourse.tile as tile
from concourse import bacc
from concourse.bass_utils import run_bass_kernel_spmd
from concourse.masks import make_identity

BF16 = ml_dtypes.bfloat16

B, S, D, H = 2, 2048, 1024, 16
HD = D // H            # 64
NH = 4                 # heads per core
JQ = NH * HD           # 256 q (or k, or v) columns per core
P = 128
SC = 512               # seq chunk (matmul free dim / PSUM bank)
NSC = S // SC          # 4
NST = S // P           # 16 seq tiles
NDC = D // P           # 8 contraction chunks over model dim
SCALE = 1.0 / np.sqrt(np.float32(HD))  # 0.125
NEG = -1.0e9

_COMPILED = {}


def build(has_qkv_bias: bool):
    f32 = mybir.dt.float32
    bf16 = mybir.dt.bfloat16
    nc = bacc.Bacc()

    xT = nc.declare_dram_parameter("xT", [D, S], bf16, isOutput=False)
    w = nc.declare_dram_parameter("w", [D, 3 * JQ], bf16, isOutput=False)
    wp = nc.declare_dram_parameter("wp", [JQ, D], bf16, isOutput=False)
    if has_qkv_bias:
        bqkv = nc.declare_dram_parameter("bqkv", [1, 3 * JQ], bf16, isOutput=False)
    y = nc.declare_dram_parameter("y", [S, D], bf16, isOutput=True)

    with tile.TileContext(nc) as tc:
        with (
            tc.tile_pool(name="const", bufs=1) as const,
            tc.tile_pool(name="inp", bufs=1) as inp,
            tc.tile_pool(name="qkv", bufs=1) as qkv,
            tc.tile_pool(name="ptp", bufs=5) as ptp,
            tc.tile_pool(name="ps", bufs=4, space="PSUM") as psp,
            tc.tile_pool(name="drp", bufs=1, space="DRAM") as drp,
        ):
            # ---- constants ----
            # -1e9 on the strict upper triangle (j < p), 0 elsewhere: added
            # into diagonal scores blocks via an identity matmul
            trineg = const.tile([P, P], bf16)
            nc.gpsimd.memset(trineg[:], 0.0)
            nc.gpsimd.affine_select(
                out=trineg[:],
                in_=trineg[:],
                compare_op=mybir.AluOpType.is_ge,
                fill=NEG,
                base=0,
                pattern=[[1, P]],
                channel_multiplier=-1,
            )
            ident = const.tile([P, P], bf16)
            make_identity(nc, ident)
            # ones rows: row 0 feeds bias matmuls, row 64 feeds the
            # denominator-broadcast matmul (lhsT/rhs must share base partition)
            ones = const.tile([P, SC], bf16)
            nc.gpsimd.memset(ones[:], 1.0)

            # ---- load inputs, split so early consumers start immediately ----
            # w tiles 0,1: Q columns; 2,3: K columns; V columns in one tile
            wt = [inp.tile([P, NDC, P], bf16, name=f"wt{j}") for j in range(4)]
            wv_sb = inp.tile([P, NDC, JQ], bf16)
            xc = [inp.tile([P, NDC, SC], bf16, name=f"xc{c}") for c in range(NSC)]

            def dma_w(j):
                nc.sync.dma_start(
                    out=wt[j][:],
                    in_=w[:, j * P:(j + 1) * P].rearrange("(a p) j2 -> p a j2", p=P),
                )

            def dma_x(c):
                nc.sync.dma_start(
                    out=xc[c][:],
                    in_=xT[:, c * SC:(c + 1) * SC].rearrange("(a p) s -> p a s", p=P),
                )

            dma_w(2)
            dma_w(0)
            # first chunk split in two so the first QKV matmul starts sooner
            nc.sync.dma_start(
                out=xc[0][:, 0:4, :],
                in_=xT[0:D // 2, 0:SC].rearrange("(a p) s -> p a s", p=P),
            )
            nc.sync.dma_start(
                out=xc[0][:, 4:NDC, :],
                in_=xT[D // 2:D, 0:SC].rearrange("(a p) s -> p a s", p=P),
            )
            nc.sync.dma_start(
                out=wv_sb[:],
                in_=w[:, 2 * JQ:3 * JQ].rearrange("(a p) j2 -> p a j2", p=P),
            )
            dma_w(3)
            dma_w(1)
            for c in range(1, NSC):
                dma_x(c)
            wp_sb = inp.tile([P, JQ // P, D], bf16)
            nc.sync.dma_start(out=wp_sb[:], in_=wp[:].rearrange("(a p) j -> p a j", p=P))
            if has_qkv_bias:
                b_sb = inp.tile([1, 3 * JQ], bf16)
                nc.sync.dma_start(out=b_sb[:], in_=bqkv[:])

            qT = qkv.tile([P, 2, S], bf16)  # partitions: head pair (h%2)*64 + hd
            kT = qkv.tile([P, 2, S], bf16)
            v_sb = qkv.tile([P, NST * NH, HD + 1], bf16)
            nc.vector.memset(v_sb[:, :, HD:HD + 1], 1.0)
            oT = qkv.tile([P, 2, S], bf16)
            dr_s = drp.tile([NH, NSC, 1, SC], f32)
            dr_r = drp.tile([NH, NSC, 1, SC], bf16)

            def emit_qk_chunk(jt, c):
                # one [128, SC] chunk of Q^T (jt 0,1) or K^T (jt 2,3)
                dest, jl = (qT, jt) if jt < 2 else (kT, jt - 2)
                ps_qkv = psp.tile([P, SC], f32, tag="ps", name="ps_qkv")
                for a in range(NDC):
                    nc.tensor.matmul(
                        ps_qkv[:],
                        lhsT=wt[jt][:, a, :],
                        rhs=xc[c][:, a, :],
                        start=(a == 0),
                        stop=(a == NDC - 1) and not has_qkv_bias,
                    )
                if has_qkv_bias:
                    nc.tensor.matmul(
                        ps_qkv[:],
                        lhsT=b_sb[0:1, jt * P:(jt + 1) * P],
                        rhs=ones[0:1, :SC],
                        start=False,
                        stop=True,
                    )
                nc.vector.tensor_copy(dest[:, jl, c * SC:(c + 1) * SC], ps_qkv[:])

            def emit_v_tile(t):
                # V rows for seq tile t, all 4 heads, with the ones column
                ps_v = psp.tile([P, SC], f32, tag="ps", name="ps_v")
                for a in range(NDC):
                    nc.tensor.matmul(
                        ps_v[:, 0:JQ],
                        lhsT=xc[t // 4][:, a, (t % 4) * P:(t % 4 + 1) * P],
                        rhs=wv_sb[:, a, :],
                        start=(a == 0),
                        stop=(a == NDC - 1) and not has_qkv_bias,
                    )
                if has_qkv_bias:
                    nc.tensor.matmul(
                        ps_v[:, 0:JQ],
                        lhsT=ones[0:1, 0:P],
                        rhs=b_sb[0:1, 2 * JQ:3 * JQ],
                        start=False,
                        stop=True,
                    )
                nc.vector.tensor_copy(
                    v_sb[:, t * NH:(t + 1) * NH, 0:HD],
                    ps_v[:, 0:JQ].rearrange("p (h d) -> p h d", d=HD),
                )

            # per-(head, chunk) softmax-denominator reciprocal chains; the
            # normalization (bcast matmul + O^T scale) is emitted a few units
            # later so the chain latency never stalls the PE
            norm_pend = []

            def emit_norm():
                h, c, recipst = norm_pend.pop(0)
                jl, po = h // 2, (h % 2) * HD
                ps_bc = psp.tile([P, SC], f32, tag="ps", name="ps_bc")
                nc.tensor.matmul(
                    ps_bc[0:HD, :],
                    lhsT=ones[64:65, 0:HD],
                    rhs=recipst[64:65, :],
                    start=True,
                    stop=True,
                )
                nc.vector.tensor_mul(
                    oT[po:po + HD, jl, c * SC:(c + 1) * SC],
                    oT[po:po + HD, jl, c * SC:(c + 1) * SC],
                    ps_bc[0:HD, :],
                )

            def emit_scores(c, h):
                jl, po = h // 2, (h % 2) * HD
                nv = min(4 * (c + 1), NST)  # valid sk tiles: t*128 <= c*512+511
                pt = ptp.tile([P, NST, SC], bf16, tag="pt", name="pt")
                for t in range(nv):
                    # first valid column within this sq chunk (causal)
                    coff = max(0, t * P - c * SC)
                    diag = t >= 4 * c
                    ps_sc = psp.tile([P, SC], f32, tag="ps_sc", name="ps_sc", bufs=4)
                    nc.tensor.matmul(
                        ps_sc[:, coff:],
                        lhsT=kT[po:po + HD, jl, t * P:(t + 1) * P],
                        rhs=qT[po:po + HD, jl, c * SC + coff:(c + 1) * SC],
                        start=True,
                        stop=not diag,
                    )
                    if diag:  # add -1e9 above the diagonal: psum += I.T @ trineg
                        nc.tensor.matmul(
                            ps_sc[:, coff:coff + P],
                            lhsT=ident[:],
                            rhs=trineg[:],
                            start=False,
                            stop=True,
                        )
                    nc.scalar.activation(
                        pt[:, t, coff:], ps_sc[:, coff:],
                        mybir.ActivationFunctionType.Exp, scale=float(SCALE),
                    )
                return (c, h, pt)

            def emit_av(state, direct_recip=False):
                c, h, pt = state
                jl, po = h // 2, (h % 2) * HD
                nv = min(4 * (c + 1), NST)
                ps_av = psp.tile([P, SC], f32, tag="ps", name="ps_av")
                for t in range(nv):
                    coff = max(0, t * P - c * SC)
                    nc.tensor.matmul(
                        ps_av[0:HD + 1, coff:],
                        lhsT=v_sb[:, t * NH + h, :],
                        rhs=pt[:, t, coff:],
                        start=(t == 0),
                        stop=(t == nv - 1),
                    )
                nc.vector.tensor_copy(
                    oT[po:po + HD, jl, c * SC:(c + 1) * SC], ps_av[0:HD, :]
                )

                # denominator reciprocal: stage the sums row, bounce through
                # DRAM to re-partition [1, SC] -> [128, SC/128] so the
                # reciprocal runs on all 128 lanes instead of one; the last
                # units take the short single-lane path instead (lower latency,
                # nothing left to hide the DMA chain behind)
                recipst = ptp.tile([P, SC], bf16, tag="recipst",
                                   name="recipst", bufs=8)
                if direct_recip:
                    with nc.allow_low_precision(reason="bf16 softmax denom recip"):
                        nc.vector.reciprocal(recipst[64:65, :], ps_av[HD:HD + 1, :])
                else:
                    sumst = ptp.tile([P, SC], f32, tag="sumst", name="sumst", bufs=3)
                    rsc = ptp.tile([P, SC // P], f32, tag="rsc", name="rsc", bufs=3)
                    rscb = ptp.tile([P, SC // P], bf16, tag="rscb", name="rscb",
                                    bufs=3)
                    nc.vector.tensor_copy(sumst[64:65, :], ps_av[HD:HD + 1, :])
                    nc.sync.dma_start(out=dr_s[h, c], in_=sumst[64:65, :])
                    nc.sync.dma_start(
                        out=rsc[:], in_=dr_s[h, c].rearrange("x (p k) -> (x p) k", p=P)
                    )
                    with nc.allow_low_precision(reason="bf16 softmax denom recip"):
                        nc.vector.reciprocal(rscb[:], rsc[:])
                    nc.sync.dma_start(
                        out=dr_r[h, c].rearrange("x (p k) -> (x p) k", p=P),
                        in_=rscb[:],
                    )
                    nc.sync.dma_start(out=recipst[64:65, :], in_=dr_r[h, c])
                norm_pend.append((h, c, recipst))
                while len(norm_pend) > 6:
                    emit_norm()

            def emit_proj(st, jc):
                ps_y = psp.tile([P, SC], f32, tag="ps", name="ps_y")
                for cc in range(2):
                    nc.tensor.matmul(
                        ps_y[:],
                        lhsT=oT[:, cc, st * P:(st + 1) * P],
                        rhs=wp_sb[:, cc, jc * SC:(jc + 1) * SC],
                        start=(cc == 0),
                        stop=(cc == 1),
                    )
                y_sb = ptp.tile([P, SC], bf16, tag="ysb", name="y_sb", bufs=4)
                nc.vector.tensor_copy(y_sb[:], ps_y[:])
                nc.sync.dma_start(
                    out=y[st * P:(st + 1) * P, jc * SC:(jc + 1) * SC], in_=y_sb[:]
                )

            # ---- main schedule ----
            # Chunk 0: produce only what each unit needs, then start it, so
            # the first exp lands on ScalarE as early as possible.
            def prod_thunks(c):
                if c >= NSC:
                    return []
                th = [lambda: emit_qk_chunk(2, c), lambda: emit_qk_chunk(0, c)]
                th += [lambda t=t: emit_v_tile(t) for t in range(4 * c, 4 * c + 4)]
                th += [lambda: emit_qk_chunk(3, c), lambda: emit_qk_chunk(1, c)]
                return th

            emit_qk_chunk(2, 0)
            emit_qk_chunk(0, 0)
            for t in range(4):
                emit_v_tile(t)
            pq = prod_thunks(1)

            def pop2():
                for _ in range(2):
                    if pq:
                        pq.pop(0)()

            # AV for each unit is emitted one unit late so the exps of a unit
            # get a whole unit of matmul time before AV consumes them
            av_pend = []

            def flush_av(keep):
                while len(av_pend) > keep:
                    st_ = av_pend.pop(0)
                    emit_av(st_, direct_recip=(st_[0] == NSC - 1 and st_[1] >= 2))

            av_pend.append(emit_scores(0, 0))
            emit_qk_chunk(3, 0)
            av_pend.append(emit_scores(0, 1))
            flush_av(1)
            emit_qk_chunk(1, 0)
            pop2()
            av_pend.append(emit_scores(0, 2))
            flush_av(1)
            pop2()
            av_pend.append(emit_scores(0, 3))
            flush_av(1)
            while pq:
                pq.pop(0)()

            # Chunks 1..3: attention units interleaved with chunk c+1's
            # production and (from chunk 2 on) with the projection of chunk
            # c-2, whose normalization completed during chunk c-1.
            for c in range(1, NSC):
                pq = prod_thunks(c + 1)
                for h in range(NH):
                    av_pend.append(emit_scores(c, h))
                    flush_av(1)
                    if c >= 2:
                        while norm_pend and norm_pend[0][1] <= c - 2:
                            emit_norm()
                        st = 4 * (c - 2) + h
                        emit_proj(st, 0)
                        emit_proj(st, 1)
                    if c == NSC - 1 and h >= 2:
                        while norm_pend and norm_pend[0][1] <= c - 1:
                            emit_norm()
                        for st in (8 + 2 * (h - 2), 9 + 2 * (h - 2)):
                            emit_proj(st, 0)
                            emit_proj(st, 1)
                    pop2()
                while pq:
                    pq.pop(0)()
            flush_av(0)

            # tail: chunk-3 projections after its norms
            while norm_pend:
                emit_norm()
            for st in range(12, NST):
                for jc in range(2):
                    emit_proj(st, jc)

    nc.compile()
    return nc


def get_compiled(has_qkv_bias: bool):
    key = bool(has_qkv_bias)
    if key not in _COMPILED:
        _COMPILED[key] = build(key)
    return _COMPILED[key]


def make_in_maps(x, Wqkv, bqkv, Wproj):
    has_bias = bool(np.any(bqkv))
    xTs = [np.ascontiguousarray(x[b].T).astype(BF16) for b in range(B)]
    in_maps = []
    for c in range(8):
        b, g = c // 4, c % 4
        sl = slice(g * JQ, (g + 1) * JQ)
        wshard = np.concatenate(
            [Wqkv[:, sl], Wqkv[:, D + g * JQ:D + (g + 1) * JQ],
             Wqkv[:, 2 * D + g * JQ:2 * D + (g + 1) * JQ]], axis=1
        ).astype(BF16)
        m = {
            "xT": xTs[b],
            "w": np.ascontiguousarray(wshard),
            "wp": np.ascontiguousarray(Wproj[sl]).astype(BF16),
        }
        if has_bias:
            bshard = np.concatenate(
                [bqkv[sl], bqkv[D + g * JQ:D + (g + 1) * JQ],
                 bqkv[2 * D + g * JQ:2 * D + (g + 1) * JQ]]
            ).astype(BF16)
            m["bqkv"] = np.ascontiguousarray(bshard[None, :])
        in_maps.append(m)
    return has_bias, in_maps


def run(x, Wqkv, bqkv, Wproj, bproj, trace=False):
    has_bias, in_maps = make_in_maps(x, Wqkv, bqkv, Wproj)
    nc = get_compiled(has_bias)
    res = run_bass_kernel_spmd(nc, in_maps, core_ids=list(range(8)), trace=trace)
    out = np.zeros((B, S, D), np.float32)
    for c in range(8):
        out[c // 4] += res.results[c]["y"].astype(np.float32)
    out += bproj.astype(np.float32)
    return out, res


def kernel(x, Wqkv, bqkv, Wproj, bproj):
    x = np.asarray(x, np.float32)
    Wqkv = np.asarray(Wqkv, np.float32)
    bqkv = np.asarray(bqkv, np.float32)
    Wproj = np.asarray(Wproj, np.float32)
    bproj = np.asarray(bproj, np.float32)
    out, _ = run(x, Wqkv, bqkv, Wproj, bproj, trace=False)
    return out
